# revision 9
# baseline (speedup 1.0000x reference)
"""Trainium2 Bass kernel for nn_CombSubFastFacV1 (DDSP CombSub vocoder).

Sharding: pure data parallel, batch row b -> NeuronCore b (8 rows, 8 cores).

Per core:
  f0 [2048] --x512 lerp upsample (exact-FMA emulation, bit-exact vs XLA-CPU)-->
  f0_up [1M]; x = cumsum(f0_up/sr) via XLA-CPU's base-16 blocked scan
  (bit-exact); combtooth = sinc(sr*wrap(x)/(f0_up+1e-3)); STFT (1024-pt,
  hop 512, sqrt-hann) of combtooth & noise as fp32r matmul DFT; complex
  spectral filters; inverse DFT matmul; overlap-add.

Self-contained: hardcodes shapes (B=8, NF=2048, BS=512, SR=44100).
"""
import numpy as np
from contextlib import ExitStack

import concourse.bass as bass
import concourse.bacc as bacc
import concourse.tile as tile
import concourse.mybir as mybir
from concourse.bass_utils import run_bass_kernel_spmd

F32 = mybir.dt.float32
F32R = mybir.dt.float32r
I32 = mybir.dt.int32
ALU = mybir.AluOpType
AF = mybir.ActivationFunctionType

P = 128
NF = 2048            # control frames
BS = 512             # block size
S = NF * BS          # 1048576 samples per row
FREE = S // P        # 8192
NBINS = 513
NBP = 514           # bins padded even for fp32r N-restriction
MT = 17              # frame tiles (16 x 128 + 1 x 1)
MAGIC = float(np.float32(1.5 * 2 ** 23))
PI = float(np.float32(np.pi))
C_INV_SR = float(np.float32(np.float32(1.0) / np.float32(44100.0)))

TRACE = False
LAST_RES = {}
_CACHE = {}


def _constants():
    n = np.arange(2 * BS)
    w = np.sqrt(0.5 * (1.0 - np.cos(2 * np.pi * n / (2 * BS))))
    k = np.arange(NBINS)
    ang = 2 * np.pi * np.outer(n, k) / (2 * BS)          # [1024, 513]
    Wc = (np.cos(ang) * w[:, None]).astype(np.float32)
    Ws = (-np.sin(ang) * w[:, None]).astype(np.float32)
    Ws[:, 512] = 0.0
    cs = np.full(NBINS, 2.0 / (2 * BS))
    cs[0] = 1.0 / (2 * BS)
    cs[512] = 1.0 / (2 * BS)
    Vc = (np.cos(ang.T) * (w[None, :] * cs[:, None])).astype(np.float32)
    Vs = (-np.sin(ang.T) * (w[None, :] * cs[:, None])).astype(np.float32)
    Vs[512, :] = 0.0
    # wc_dev[p, ks*514 + c] = Wc[ks*128+p, c], bins padded 513->514 (even N for fp32r)
    Wc2 = np.zeros((8 * P, NBP), np.float32)
    Wc2[:, :NBINS] = Wc
    Ws2 = np.zeros((8 * P, NBP), np.float32)
    Ws2[:, :NBINS] = Ws
    wc_dev = Wc2.reshape(8, P, NBP).transpose(1, 0, 2).reshape(P, 8 * NBP).copy()
    ws_dev = Ws2.reshape(8, P, NBP).transpose(1, 0, 2).reshape(P, 8 * NBP).copy()
    # vc_dev[p, kb*1024 + nn] = Vc_pad[kb*128+p, nn], bins padded to 640
    Vc_pad = np.zeros((5 * P, 2 * BS), np.float32)
    Vc_pad[:NBINS] = Vc
    Vs_pad = np.zeros((5 * P, 2 * BS), np.float32)
    Vs_pad[:NBINS] = Vs
    vc_dev = Vc_pad.reshape(5, P, 2 * BS).transpose(1, 0, 2).reshape(P, 10 * BS).copy()
    vs_dev = Vs_pad.reshape(5, P, 2 * BS).transpose(1, 0, 2).reshape(P, 10 * BS).copy()
    eye = np.eye(P, dtype=np.float32)
    return wc_dev, ws_dev, vc_dev, vs_dev, eye


def _emit(nc, tc, ctx, aps):
    f0_d, hm_d, hp_d, nm_d, nu_d, wc_d, ws_d, vc_d, vs_d, eye_d, out_d = aps
    v = nc.vector
    sc = nc.scalar
    gp = nc.gpsimd

    # whole-kernel pools
    cpool = ctx.enter_context(tc.tile_pool(name="consts", bufs=1))
    persist = ctx.enter_context(tc.tile_pool(name="persist", bufs=1))
    ps_tr = ctx.enter_context(tc.tile_pool(name="pstr", bufs=2, space="PSUM"))
    ps_fw = ctx.enter_context(tc.tile_pool(name="psfw", bufs=4, space="PSUM"))
    ps_iv = ctx.enter_context(tc.tile_pool(name="psiv", bufs=2, space="PSUM"))

    # ---------------- small constants ----------------
    eye = cpool.tile([P, P], F32, tag="eye")
    nc.sync.dma_start(eye[:], eye_d[:])
    it = cpool.tile([P, BS], I32, tag="iota")
    gp.iota(it[:], pattern=[[1, BS]], base=0, channel_multiplier=0)
    frac = cpool.tile([P, BS], F32, tag="frac")
    v.tensor_copy(frac[:], it[:])
    v.tensor_scalar(frac[:], frac[:], float(np.float32(1.0 / 512.0)), None, ALU.mult)
    utab = cpool.tile([P, BS], F32, tag="utab")
    v.tensor_scalar(utab[:], frac[:], -1.0, 1.0, ALU.mult, ALU.add)
    fe = cpool.tile([P, 17], F32, tag="fe")
    nc.sync.dma_start(fe[:, 0:16], f0_d.rearrange("(p j) -> p j", p=P))
    nc.sync.dma_start(fe[0:127, 16:17], f0_d[16:2048:16])
    nc.sync.dma_start(fe[127:128, 16:17], f0_d[2047:2048])
    ahi = cpool.tile([P, 17], F32, tag="ahi")
    v.tensor_scalar(ahi[:, 0:16].bitcast(I32), fe[:, 0:16].bitcast(I32),
                    -1024, None, ALU.bitwise_and)
    alo = cpool.tile([P, 17], F32, tag="alo")
    v.tensor_tensor(alo[:, 0:16], fe[:, 0:16], ahi[:, 0:16], ALU.subtract)

    # persistent big signals
    zeros128 = cpool.tile([P, P], F32, tag="zeros128")
    v.memset(zeros128[:], 0.0)
    noise_cm = persist.tile([P, FREE + 8], F32R, tag="noiseCM")
    xt = persist.tile([P, FREE + 8], F32R, tag="toothCM")
    v.tensor_copy(noise_cm[:, 0:4], zeros128[:, 0:4])
    v.tensor_copy(noise_cm[:, FREE + 4:FREE + 8], zeros128[:, 0:4])
    v.tensor_copy(xt[:, 0:4], zeros128[:, 0:4])
    v.tensor_copy(xt[:, FREE + 4:FREE + 8], zeros128[:, 0:4])

    # ---------------- noise: DMA + transpose + scale ----------------
    with tc.tile_pool(name="nrm", bufs=1) as nrm_pool:
        noise_rm = nrm_pool.tile([P, FREE], F32, tag="nrm")
        nc.sync.dma_start(noise_rm[:], nu_d.rearrange("(p f) -> p f", p=P))
        for m in range(64):
            pt = ps_tr.tile([P, P], F32, tag="tr")
            nc.tensor.transpose(pt[:], noise_rm[:, m * P:(m + 1) * P], eye[:])
            sc.activation(noise_cm[:, 4 + m:4 + m + 64 * (P - 1) + 1:64], pt[:],
                          AF.Copy, bias=-float(2.0 ** -7), scale=float(2.0 ** -6))

    # ---------------- phase path ----------------
    with tc.tile_pool(name="f0cmp", bufs=1) as f0cm_pool:
        with tc.tile_pool(name="fx", bufs=1) as fx_pool:
            f0up = fx_pool.tile([P, FREE], F32, tag="f0up")
            X = fx_pool.tile([P, FREE], F32, tag="X")
            # P1: f0_up exact-FMA lerp + increments
            with tc.tile_pool(name="p1", bufs=2) as p1p:
                for k in range(16):
                    e = gp if (k % 2 == 1) else v
                    blk = slice(k * BS, (k + 1) * BS)
                    t_ = p1p.tile([P, BS], F32, tag="t")
                    v.tensor_scalar(t_[:], frac[:], fe[:, k + 1:k + 2], None, ALU.mult)
                    q1 = p1p.tile([P, BS], F32, tag="q1")
                    v.tensor_scalar(q1[:], utab[:], ahi[:, k:k + 1], None, ALU.mult)
                    q2 = p1p.tile([P, BS], F32, tag="q2")
                    v.tensor_scalar(q2[:], utab[:], alo[:, k:k + 1], None, ALU.mult)
                    s_ = p1p.tile([P, BS], F32, tag="s")
                    e.tensor_tensor(s_[:], t_[:], q1[:], ALU.add)
                    ap_ = p1p.tile([P, BS], F32, tag="ap")
                    e.tensor_tensor(ap_[:], s_[:], q1[:], ALU.subtract)
                    bp_ = p1p.tile([P, BS], F32, tag="bp")
                    e.tensor_tensor(bp_[:], s_[:], ap_[:], ALU.subtract)
                    e.tensor_tensor(ap_[:], t_[:], ap_[:], ALU.subtract)
                    e.tensor_tensor(bp_[:], q1[:], bp_[:], ALU.subtract)
                    e.tensor_tensor(ap_[:], ap_[:], bp_[:], ALU.add)
                    e.tensor_tensor(ap_[:], q2[:], ap_[:], ALU.add)
                    e.tensor_tensor(f0up[:, blk], s_[:], ap_[:], ALU.add)
                    v.tensor_scalar(X[:, blk], f0up[:, blk], C_INV_SR, None, ALU.mult)

            # P2: base-16 blocked scan (bit-exact vs XLA-CPU)
            with tc.tile_pool(name="scan", bufs=1) as sp:
                for j in range(1, 16):
                    v.tensor_tensor(X[:, j::16], X[:, j::16], X[:, j - 1::16], ALU.add)
                S0 = sp.tile([P, 512], F32, tag="S0")
                v.tensor_copy(S0[:], X[:, 15::16])
                for j in range(1, 16):
                    v.tensor_tensor(S0[:, j::16], S0[:, j::16], S0[:, j - 1::16], ALU.add)
                S1 = sp.tile([P, 32], F32, tag="S1")
                v.tensor_copy(S1[:], S0[:, 15::16])
                for j in range(1, 16):
                    v.tensor_tensor(S1[:, j::16], S1[:, j::16], S1[:, j - 1::16], ALU.add)
                t256 = sp.tile([1, 256], F32, tag="t256")
                nc.sync.dma_start(t256[:], S1[:, 15::16])
                for j in range(1, 16):
                    v.tensor_tensor(t256[:, j::16], t256[:, j::16],
                                    t256[:, j - 1::16], ALU.add)
                t4 = sp.tile([1, 16], F32, tag="t4")
                v.tensor_tensor_scan(t4[:], t256[:, 15::16], t256[:, 15::16], 0.0,
                                     ALU.add, ALU.bypass)
                for m in range(1, 16):
                    v.tensor_scalar(t256[:, 16 * m:16 * m + 16],
                                    t256[:, 16 * m:16 * m + 16],
                                    t4[:, m - 1:m], None, ALU.add)
                off2 = sp.tile([P, 2], F32, tag="off2")
                gp.memset(off2[0:1, 0:1], 0.0)
                nc.sync.dma_start(off2[0:1, 1:2], t256[0:1, 0:1])
                nc.sync.dma_start(off2[1:128, 0:2], t256[0:1, 1:255])
                v.tensor_scalar(S1[:, 0:16], S1[:, 0:16], off2[:, 0:1], None, ALU.add)
                v.tensor_scalar(S1[:, 16:32], S1[:, 16:32], off2[:, 1:2], None, ALU.add)
                sh1 = sp.tile([P, 32], F32, tag="sh1")
                gp.memset(sh1[0:1, 0:1], 0.0)
                v.tensor_copy(sh1[:, 1:32], S1[:, 0:31])
                nc.sync.dma_start(sh1[1:128, 0:1], S1[0:127, 31:32])
                for t in range(16):
                    v.tensor_tensor(S0[:, t::16], S0[:, t::16], sh1[:], ALU.add)
                sh0 = sp.tile([P, 512], F32, tag="sh0")
                gp.memset(sh0[0:1, 0:1], 0.0)
                v.tensor_copy(sh0[:, 1:512], S0[:, 0:511])
                nc.sync.dma_start(sh0[1:128, 0:1], S0[0:127, 511:512])
                for t in range(16):
                    v.tensor_tensor(X[:, t::16], X[:, t::16], sh0[:], ALU.add)

            # P4: wrap + sinc in ROW-MAJOR (elementwise), tooth written
            # in-place into X (f32) block by block
            JB = 2048
            with tc.tile_pool(name="p4", bufs=1) as p4p:
                for jb in range(4):
                    e = gp if (jb % 2 == 1) else v
                    c0 = jb * JB
                    xsl = X[:, c0:c0 + JB]
                    fsl = f0up[:, c0:c0 + JB]
                    A = p4p.tile([P, JB], F32, tag="A")
                    Bt = p4p.tile([P, JB], F32, tag="B")
                    Ct = p4p.tile([P, JB], F32, tag="C")
                    Dt = p4p.tile([P, JB], F32, tag="D")
                    v.tensor_scalar(A[:], xsl, MAGIC, None, ALU.add)
                    v.tensor_scalar(A[:], A[:], MAGIC, None, ALU.subtract)
                    e.tensor_tensor(A[:], xsl, A[:], ALU.subtract)            # xw
                    v.tensor_scalar(Bt[:], fsl, 0.001, None, ALU.add)         # den
                    v.reciprocal_approx_accurate(Ct[:], Bt[:], scratch=Dt[:])
                    v.tensor_scalar(A[:], A[:], 44100.0, None, ALU.mult)      # num
                    e.tensor_tensor(A[:], A[:], Ct[:], ALU.mult)              # v
                    v.tensor_scalar(Bt[:], A[:], 0.5, None, ALU.mult)
                    v.tensor_scalar(Bt[:], Bt[:], MAGIC, None, ALU.add)
                    v.tensor_scalar(Bt[:], Bt[:], MAGIC, None, ALU.subtract)
                    v.scalar_tensor_tensor(Bt[:], Bt[:], -2.0, A[:],
                                           ALU.mult, ALU.add)                 # w
                    sc.activation(Ct[:], Bt[:], AF.Sin, scale=PI)             # sin(pi w)
                    v.tensor_scalar(Bt[:], A[:], PI, None, ALU.mult)          # pi v
                    v.tensor_scalar(Dt[:], Bt[:], 0.0, None, ALU.is_equal)    # m
                    e.tensor_tensor(Bt[:], Bt[:], Dt[:], ALU.add)             # piv + m
                    e.tensor_tensor(Ct[:], Ct[:], Dt[:], ALU.add)             # sin + m
                    v.reciprocal_approx_accurate(A[:], Bt[:], scratch=Dt[:])
                    e.tensor_tensor(xsl, Ct[:], A[:], ALU.mult)               # tooth

            # P3: transpose tooth to column-major f32r
            for m in range(64):
                pt = ps_tr.tile([P, P], F32, tag="tr")
                nc.tensor.transpose(pt[:], X[:, m * P:(m + 1) * P], eye[:])
                sc.copy(xt[:, 4 + m:4 + m + 64 * (P - 1) + 1:64], pt[:])
        # fx_pool closed: f0up/X freed

    # ---------------- F: forward DFT, filter, inverse DFT, OLA ----------
    with ExitStack() as fctx:
        fcons = fctx.enter_context(tc.tile_pool(name="fcons", bufs=1))
        fstr = fctx.enter_context(tc.tile_pool(name="fstream", bufs=1))
        olap = fctx.enter_context(tc.tile_pool(name="ola", bufs=2))
        wc = fcons.tile([P, 8 * NBP], F32R, tag="wc")
        nc.sync.dma_start(wc[:], wc_d[:].bitcast(F32R))
        wss = fcons.tile([P, 8 * NBP], F32R, tag="ws")
        nc.sync.dma_start(wss[:], ws_d[:].bitcast(F32R))
        vc = fcons.tile([P, 10 * BS], F32R, tag="vc")
        nc.sync.dma_start(vc[:], vc_d[:].bitcast(F32R))
        vs = fcons.tile([P, 10 * BS], F32R, tag="vs")
        nc.sync.dma_start(vs[:], vs_d[:].bitcast(F32R))

        out_v = out_d.rearrange("(t p r) -> t p r", t=16, p=P)
        prev_h1 = None
        prev_h2 = None
        NB = ((0, 258), (258, 256))
        for mt in range(MT):
            M = P if mt < 16 else 1
            r0, r1 = (mt * P, mt * P + M) if mt < 16 else (NF - 1, NF)
            hmt = fstr.tile([M, NBINS], F32, tag="hm")
            hpt = fstr.tile([M, NBINS], F32, tag="hp")
            nmt = fstr.tile([M, NBINS], F32, tag="nm")
            nc.sync.dma_start(hmt[:], hm_d[r0:r1, :])
            nc.sync.dma_start(hpt[:], hp_d[r0:r1, :])
            nc.sync.dma_start(nmt[:], nm_d[r0:r1, :])
            ee = fstr.tile([M, NBINS], F32, tag="ee")
            sc.activation(ee[:], hmt[:], AF.Exp)
            en = fstr.tile([M, NBINS], F32, tag="en")
            sc.activation(en[:], nmt[:], AF.Exp)
            zz = fstr.tile([M, NBINS], F32, tag="zz")
            v.tensor_scalar(zz[:], hpt[:], 0.5, None, ALU.mult)
            v.tensor_scalar(zz[:], zz[:], MAGIC, None, ALU.add)
            v.tensor_scalar(zz[:], zz[:], MAGIC, None, ALU.subtract)
            nc.vector.scalar_tensor_tensor(zz[:], zz[:], -2.0, hpt[:],
                                           ALU.mult, ALU.add)
            ssn = fstr.tile([M, NBINS], F32, tag="ssn")
            sc.activation(ssn[:], zz[:], AF.Sin, scale=PI)
            v.tensor_scalar(zz[:], zz[:], 0.5, None, ALU.add)
            mg = fstr.tile([M, NBINS], F32, tag="mg")
            v.tensor_scalar(mg[:], zz[:], 1.0, None, ALU.is_gt)
            nc.vector.scalar_tensor_tensor(zz[:], mg[:], -2.0, zz[:],
                                           ALU.mult, ALU.add)
            ccn = fstr.tile([M, NBINS], F32, tag="ccn")
            sc.activation(ccn[:], zz[:], AF.Sin, scale=PI)
            fR = fstr.tile([M, NBINS], F32, tag="fR")
            gp.tensor_tensor(fR[:], ee[:], ccn[:], ALU.mult)
            fI = fstr.tile([M, NBINS], F32, tag="fI")
            gp.tensor_tensor(fI[:], ee[:], ssn[:], ALU.mult)

            xr = fstr.tile([M, NBINS], F32, tag="xr")
            xi = fstr.tile([M, NBINS], F32, tag="xi")
            tmpc = fstr.tile([M, NBINS], F32, tag="tmpc")
            for sig_i, sig_t in ((0, xt), (1, noise_cm)):
                for b0, bw in NB:
                    cw = min(b0 + bw, NBINS) - b0   # clip to real bins
                    osl = slice(b0, b0 + cw)
                    pf = ps_fw.tile([M, 258], F32, tag="fw")
                    for ks in range(8):
                        lhs = sig_t[:, 512 * mt + ks:512 * mt + ks + 4 * (M - 1) + 1:4]
                        nc.tensor.matmul(
                            pf[:, 0:bw], lhs,
                            wc[:, ks * NBP + b0:ks * NBP + b0 + bw],
                            start=(ks == 0), stop=(ks == 7))
                    if sig_i == 0:
                        v.tensor_tensor(xr[:, osl], pf[:, 0:cw], fR[:, osl], ALU.mult)
                        v.tensor_tensor(xi[:, osl], pf[:, 0:cw], fI[:, osl], ALU.mult)
                    else:
                        v.tensor_tensor(tmpc[:, osl], pf[:, 0:cw], en[:, osl], ALU.mult)
                        v.tensor_tensor(xr[:, osl], xr[:, osl], tmpc[:, osl], ALU.add)
                    pfs = ps_fw.tile([M, 258], F32, tag="fw")
                    for ks in range(8):
                        lhs = sig_t[:, 512 * mt + ks:512 * mt + ks + 4 * (M - 1) + 1:4]
                        nc.tensor.matmul(
                            pfs[:, 0:bw], lhs,
                            wss[:, ks * NBP + b0:ks * NBP + b0 + bw],
                            start=(ks == 0), stop=(ks == 7))
                    if sig_i == 0:
                        v.tensor_tensor(tmpc[:, osl], pfs[:, 0:cw], fI[:, osl], ALU.mult)
                        v.tensor_tensor(xr[:, osl], xr[:, osl], tmpc[:, osl],
                                        ALU.subtract)
                        v.tensor_tensor(tmpc[:, osl], pfs[:, 0:cw], fR[:, osl], ALU.mult)
                        v.tensor_tensor(xi[:, osl], xi[:, osl], tmpc[:, osl], ALU.add)
                    else:
                        v.tensor_tensor(tmpc[:, osl], pfs[:, 0:cw], en[:, osl], ALU.mult)
                        v.tensor_tensor(xi[:, osl], xi[:, osl], tmpc[:, osl], ALU.add)

            # transpose XR, XI -> [bins(pad 640), frames]
            xrt = fstr.tile([P, 5 * P], F32R, tag="xrt")
            xit = fstr.tile([P, 5 * P], F32R, tag="xit")
            v.tensor_copy(xrt[:, 512:640], zeros128[:])
            v.tensor_copy(xit[:, 512:640], zeros128[:])
            for kb in range(4):
                pt = ps_tr.tile([P, P], F32, tag="tr")
                nc.tensor.transpose(pt[0:P, 0:M], xr[:, kb * P:(kb + 1) * P],
                                    eye[0:M, 0:M])
                sc.copy(xrt[:, kb * P:kb * P + M], pt[0:P, 0:M])
                pt2 = ps_tr.tile([P, P], F32, tag="tr")
                nc.tensor.transpose(pt2[0:P, 0:M], xi[:, kb * P:(kb + 1) * P],
                                    eye[0:M, 0:M])
                sc.copy(xit[:, kb * P:kb * P + M], pt2[0:P, 0:M])
            ptn = ps_tr.tile([P, P], F32, tag="tr")
            nc.tensor.transpose(ptn[0:1, 0:M], xr[:, 512:513], eye[0:M, 0:M])
            sc.copy(xrt[0:1, 512:512 + M], ptn[0:1, 0:M])
            ptn2 = ps_tr.tile([P, P], F32, tag="tr")
            nc.tensor.transpose(ptn2[0:1, 0:M], xi[:, 512:513], eye[0:M, 0:M])
            sc.copy(xit[0:1, 512:512 + M], ptn2[0:1, 0:M])

            # inverse matmuls: F[frames, 1024] in two PSUM halves
            Fh = []
            for h in range(2):
                pv = ps_iv.tile([M, BS], F32, tag="iv")
                for kb in range(5):
                    nc.tensor.matmul(
                        pv[:], xrt[:, kb * P:kb * P + M],
                        vc[:, kb * 2 * BS + h * BS:kb * 2 * BS + (h + 1) * BS],
                        start=(kb == 0), stop=False)
                    nc.tensor.matmul(
                        pv[:], xit[:, kb * P:kb * P + M],
                        vs[:, kb * 2 * BS + h * BS:kb * 2 * BS + (h + 1) * BS],
                        start=False, stop=(kb == 4))
                Fh.append(pv)

            # overlap-add
            h1f = olap.tile([M, BS], F32, tag="h1f")
            sc.copy(h1f[:], Fh[0][:])
            if mt > 0:
                nc.sync.dma_start(prev_h1[127:128, :], h1f[0:1, :])
                ob = olap.tile([P, BS], F32, tag="ob")
                v.tensor_tensor(ob[:], prev_h1[:], prev_h2[:], ALU.add)
                nc.sync.dma_start(out_v[mt - 1], ob[:])
            if mt < 16:
                h1s = olap.tile([P, BS], F32, tag="h1s")
                h2s = olap.tile([P, BS], F32, tag="h2s")
                nc.sync.dma_start(h1s[0:127, :], h1f[1:128, :])
                sc.copy(h2s[:], Fh[1][:])
                prev_h1, prev_h2 = h1s, h2s


def _build():
    if "nc" in _CACHE:
        return _CACHE["nc"]
    nc = bacc.Bacc("TRN2", target_bir_lowering=False, debug=False, num_devices=8)
    f0_d = nc.dram_tensor("f0", [NF], F32, kind="ExternalInput").ap()
    hm_d = nc.dram_tensor("hm", [NF, NBINS], F32, kind="ExternalInput").ap()
    hp_d = nc.dram_tensor("hp", [NF, NBINS], F32, kind="ExternalInput").ap()
    nm_d = nc.dram_tensor("nm", [NF, NBINS], F32, kind="ExternalInput").ap()
    nu_d = nc.dram_tensor("nu", [S], F32, kind="ExternalInput").ap()
    wc_d = nc.dram_tensor("wcc", [P, 8 * NBP], F32, kind="ExternalInput").ap()
    ws_d = nc.dram_tensor("wsc", [P, 8 * NBP], F32, kind="ExternalInput").ap()
    vc_d = nc.dram_tensor("vcc", [P, 10 * BS], F32, kind="ExternalInput").ap()
    vs_d = nc.dram_tensor("vsc", [P, 10 * BS], F32, kind="ExternalInput").ap()
    eye_d = nc.dram_tensor("eye", [P, P], F32, kind="ExternalInput").ap()
    out_d = nc.dram_tensor("out", [S], F32, kind="ExternalOutput").ap()
    with tile.TileContext(nc) as tc:
        with ExitStack() as ctx:
            _emit(nc, tc, ctx, (f0_d, hm_d, hp_d, nm_d, nu_d,
                                wc_d, ws_d, vc_d, vs_d, eye_d, out_d))
    nc.compile()
    _CACHE["nc"] = nc
    return nc


def kernel(**inputs):
    f0 = np.ascontiguousarray(np.asarray(inputs["f0_frames"], np.float32))
    hm = np.ascontiguousarray(np.asarray(inputs["harmonic_magnitude"], np.float32))
    hp = np.ascontiguousarray(np.asarray(inputs["harmonic_phase"], np.float32))
    nm = np.ascontiguousarray(np.asarray(inputs["noise_magnitude"], np.float32))
    nu = np.ascontiguousarray(np.asarray(inputs["noise_u"], np.float32))
    assert int(inputs["sampling_rate"]) == 44100 and int(inputs["block_size"]) == BS
    assert f0.shape == (8, NF) and nu.shape == (8, S)

    wc_dev, ws_dev, vc_dev, vs_dev, eye = _constants()
    nc = _build()
    in_maps = []
    for b in range(8):
        in_maps.append({
            "f0": f0[b], "hm": hm[b], "hp": hp[b], "nm": nm[b], "nu": nu[b],
            "wcc": wc_dev, "wsc": ws_dev, "vcc": vc_dev, "vsc": vs_dev, "eye": eye,
        })
    res = run_bass_kernel_spmd(nc, in_maps, list(range(8)), trace=TRACE)
    LAST_RES["res"] = res
    out = np.stack([res.results[b]["out"] for b in range(8)]).astype(np.float32)
    return out


# revision 10
# speedup vs baseline: 1.2724x; 1.2724x over previous
"""Trainium2 Bass kernel for nn_CombSubFastFacV1 (DDSP CombSub vocoder).

Sharding: pure data parallel, batch row b -> NeuronCore b (8 rows, 8 cores).

Per core:
  f0 [2048] --x512 lerp upsample (exact-FMA emulation, bit-exact vs XLA-CPU)-->
  f0_up [1M]; x = cumsum(f0_up/sr) via XLA-CPU's base-16 blocked scan
  (bit-exact); combtooth = sinc(sr*wrap(x)/(f0_up+1e-3)); STFT (1024-pt,
  hop 512, sqrt-hann) of combtooth & noise as fp32r matmul DFT; complex
  spectral filters; inverse DFT matmul; overlap-add.

Self-contained: hardcodes shapes (B=8, NF=2048, BS=512, SR=44100).
"""
import numpy as np
from contextlib import ExitStack

import concourse.bass as bass
import concourse.bacc as bacc
import concourse.tile as tile
import concourse.mybir as mybir
from concourse.bass_utils import run_bass_kernel_spmd

F32 = mybir.dt.float32
F32R = mybir.dt.float32r
I32 = mybir.dt.int32
ALU = mybir.AluOpType
AF = mybir.ActivationFunctionType

P = 128
NF = 2048            # control frames
BS = 512             # block size
S = NF * BS          # 1048576 samples per row
FREE = S // P        # 8192
NBINS = 513
NBP = 514           # bins padded even for fp32r N-restriction
MT = 17              # frame tiles (16 x 128 + 1 x 1)
MAGIC = float(np.float32(1.5 * 2 ** 23))
PI = float(np.float32(np.pi))
C_INV_SR = float(np.float32(np.float32(1.0) / np.float32(44100.0)))

TRACE = False
LAST_RES = {}
_CACHE = {}


def _constants():
    n = np.arange(2 * BS)
    w = np.sqrt(0.5 * (1.0 - np.cos(2 * np.pi * n / (2 * BS))))
    k = np.arange(NBINS)
    ang = 2 * np.pi * np.outer(n, k) / (2 * BS)          # [1024, 513]
    Wc = (np.cos(ang) * w[:, None]).astype(np.float32)
    Ws = (-np.sin(ang) * w[:, None]).astype(np.float32)
    Ws[:, 512] = 0.0
    cs = np.full(NBINS, 2.0 / (2 * BS))
    cs[0] = 1.0 / (2 * BS)
    cs[512] = 1.0 / (2 * BS)
    Vc = (np.cos(ang.T) * (w[None, :] * cs[:, None])).astype(np.float32)
    Vs = (-np.sin(ang.T) * (w[None, :] * cs[:, None])).astype(np.float32)
    Vs[512, :] = 0.0
    # wc_dev[p, ks*514 + c] = Wc[ks*128+p, c], bins padded 513->514 (even N for fp32r)
    Wc2 = np.zeros((8 * P, NBP), np.float32)
    Wc2[:, :NBINS] = Wc
    Ws2 = np.zeros((8 * P, NBP), np.float32)
    Ws2[:, :NBINS] = Ws
    wc_dev = Wc2.reshape(8, P, NBP).transpose(1, 0, 2).reshape(P, 8 * NBP).copy()
    ws_dev = Ws2.reshape(8, P, NBP).transpose(1, 0, 2).reshape(P, 8 * NBP).copy()
    # vc_dev[p, kb*1024 + nn] = Vc_pad[kb*128+p, nn], bins padded to 640
    Vc_pad = np.zeros((5 * P, 2 * BS), np.float32)
    Vc_pad[:NBINS] = Vc
    Vs_pad = np.zeros((5 * P, 2 * BS), np.float32)
    Vs_pad[:NBINS] = Vs
    vc_dev = Vc_pad.reshape(5, P, 2 * BS).transpose(1, 0, 2).reshape(P, 10 * BS).copy()
    vs_dev = Vs_pad.reshape(5, P, 2 * BS).transpose(1, 0, 2).reshape(P, 10 * BS).copy()
    eye = np.eye(P, dtype=np.float32)
    return wc_dev, ws_dev, vc_dev, vs_dev, eye


def _emit(nc, tc, ctx, aps):
    f0_d, hm_d, hp_d, nm_d, nu_d, wc_d, ws_d, vc_d, vs_d, eye_d, out_d = aps
    v = nc.vector
    sc = nc.scalar
    gp = nc.gpsimd

    # whole-kernel pools
    cpool = ctx.enter_context(tc.tile_pool(name="consts", bufs=1))
    persist = ctx.enter_context(tc.tile_pool(name="persist", bufs=1))
    ps_tr = ctx.enter_context(tc.tile_pool(name="pstr", bufs=2, space="PSUM"))
    ps_fw = ctx.enter_context(tc.tile_pool(name="psfw", bufs=4, space="PSUM"))
    ps_iv = ctx.enter_context(tc.tile_pool(name="psiv", bufs=2, space="PSUM"))

    # ---------------- small constants ----------------
    eye = cpool.tile([P, P], F32, tag="eye")
    nc.sync.dma_start(eye[:], eye_d[:])
    it = cpool.tile([P, BS], I32, tag="iota")
    gp.iota(it[:], pattern=[[1, BS]], base=0, channel_multiplier=0)
    frac = cpool.tile([P, BS], F32, tag="frac")
    v.tensor_copy(frac[:], it[:])
    v.tensor_scalar(frac[:], frac[:], float(np.float32(1.0 / 512.0)), None, ALU.mult)
    utab = cpool.tile([P, BS], F32, tag="utab")
    v.tensor_scalar(utab[:], frac[:], -1.0, 1.0, ALU.mult, ALU.add)
    fe = cpool.tile([P, 17], F32, tag="fe")
    nc.sync.dma_start(fe[:, 0:16], f0_d.rearrange("(p j) -> p j", p=P))
    nc.sync.dma_start(fe[0:127, 16:17], f0_d[16:2048:16])
    nc.sync.dma_start(fe[127:128, 16:17], f0_d[2047:2048])
    ahi = cpool.tile([P, 17], F32, tag="ahi")
    v.tensor_scalar(ahi[:, 0:16].bitcast(I32), fe[:, 0:16].bitcast(I32),
                    -1024, None, ALU.bitwise_and)
    alo = cpool.tile([P, 17], F32, tag="alo")
    v.tensor_tensor(alo[:, 0:16], fe[:, 0:16], ahi[:, 0:16], ALU.subtract)

    # persistent big signals
    zeros128 = cpool.tile([P, P], F32, tag="zeros128")
    v.memset(zeros128[:], 0.0)
    noise_cm = persist.tile([P, FREE + 8], F32R, tag="noiseCM")
    xt = persist.tile([P, FREE + 8], F32R, tag="toothCM")
    v.tensor_copy(noise_cm[:, 0:4], zeros128[:, 0:4])
    v.tensor_copy(noise_cm[:, FREE + 4:FREE + 8], zeros128[:, 0:4])
    v.tensor_copy(xt[:, 0:4], zeros128[:, 0:4])
    v.tensor_copy(xt[:, FREE + 4:FREE + 8], zeros128[:, 0:4])

    # ---------------- noise: DMA + transpose + scale ----------------
    with tc.tile_pool(name="nrm", bufs=1) as nrm_pool:
        noise_rm = nrm_pool.tile([P, FREE], F32, tag="nrm")
        nc.sync.dma_start(noise_rm[:], nu_d.rearrange("(p f) -> p f", p=P))
        for m in range(64):
            pt = ps_tr.tile([P, P], F32, tag="tr")
            nc.tensor.transpose(pt[:], noise_rm[:, m * P:(m + 1) * P], eye[:])
            v.tensor_scalar(noise_cm[:, 4 + m:4 + m + 64 * (P - 1) + 1:64], pt[:],
                            float(2.0 ** -6), float(2.0 ** -7),
                            ALU.mult, ALU.subtract)

    # ---------------- phase path ----------------
    with tc.tile_pool(name="f0cmp", bufs=1) as f0cm_pool:
        with tc.tile_pool(name="fx", bufs=1) as fx_pool:
            f0up = fx_pool.tile([P, FREE], F32, tag="f0up")
            X = fx_pool.tile([P, FREE], F32, tag="X")
            # P1: f0_up exact-FMA lerp + increments
            with tc.tile_pool(name="p1", bufs=2) as p1p:
                for k in range(16):
                    blk = slice(k * BS, (k + 1) * BS)
                    t_ = p1p.tile([P, BS], F32, tag="t")
                    v.tensor_scalar(t_[:], frac[:], fe[:, k + 1:k + 2], None, ALU.mult)
                    q1 = p1p.tile([P, BS], F32, tag="q1")
                    v.tensor_scalar(q1[:], utab[:], ahi[:, k:k + 1], None, ALU.mult)
                    q2 = p1p.tile([P, BS], F32, tag="q2")
                    v.tensor_scalar(q2[:], utab[:], alo[:, k:k + 1], None, ALU.mult)
                    s_ = p1p.tile([P, BS], F32, tag="s")
                    v.tensor_tensor(s_[:], t_[:], q1[:], ALU.add)
                    ap_ = p1p.tile([P, BS], F32, tag="ap")
                    v.tensor_tensor(ap_[:], s_[:], q1[:], ALU.subtract)
                    bp_ = p1p.tile([P, BS], F32, tag="bp")
                    v.tensor_tensor(bp_[:], s_[:], ap_[:], ALU.subtract)
                    v.tensor_tensor(ap_[:], t_[:], ap_[:], ALU.subtract)
                    v.tensor_tensor(bp_[:], q1[:], bp_[:], ALU.subtract)
                    v.tensor_tensor(ap_[:], ap_[:], bp_[:], ALU.add)
                    v.tensor_tensor(ap_[:], q2[:], ap_[:], ALU.add)
                    v.tensor_tensor(f0up[:, blk], s_[:], ap_[:], ALU.add)
                    v.tensor_scalar(X[:, blk], f0up[:, blk], C_INV_SR, None, ALU.mult)

            # P2: base-16 blocked scan (bit-exact vs XLA-CPU)
            with tc.tile_pool(name="scan", bufs=1) as sp:
                for j in range(1, 16):
                    v.tensor_tensor(X[:, j::16], X[:, j::16], X[:, j - 1::16], ALU.add)
                S0 = sp.tile([P, 512], F32, tag="S0")
                v.tensor_copy(S0[:], X[:, 15::16])
                for j in range(1, 16):
                    v.tensor_tensor(S0[:, j::16], S0[:, j::16], S0[:, j - 1::16], ALU.add)
                S1 = sp.tile([P, 32], F32, tag="S1")
                v.tensor_copy(S1[:], S0[:, 15::16])
                for j in range(1, 16):
                    v.tensor_tensor(S1[:, j::16], S1[:, j::16], S1[:, j - 1::16], ALU.add)
                t256 = sp.tile([1, 256], F32, tag="t256")
                nc.sync.dma_start(t256[:], S1[:, 15::16])
                for j in range(1, 16):
                    v.tensor_tensor(t256[:, j::16], t256[:, j::16],
                                    t256[:, j - 1::16], ALU.add)
                t4 = sp.tile([1, 16], F32, tag="t4")
                v.tensor_tensor_scan(t4[:], t256[:, 15::16], t256[:, 15::16], 0.0,
                                     ALU.add, ALU.bypass)
                for m in range(1, 16):
                    v.tensor_scalar(t256[:, 16 * m:16 * m + 16],
                                    t256[:, 16 * m:16 * m + 16],
                                    t4[:, m - 1:m], None, ALU.add)
                off2 = sp.tile([P, 2], F32, tag="off2")
                gp.memset(off2[0:1, 0:1], 0.0)
                nc.sync.dma_start(off2[0:1, 1:2], t256[0:1, 0:1])
                nc.sync.dma_start(off2[1:128, 0:2], t256[0:1, 1:255])
                v.tensor_scalar(S1[:, 0:16], S1[:, 0:16], off2[:, 0:1], None, ALU.add)
                v.tensor_scalar(S1[:, 16:32], S1[:, 16:32], off2[:, 1:2], None, ALU.add)
                sh1 = sp.tile([P, 32], F32, tag="sh1")
                gp.memset(sh1[0:1, 0:1], 0.0)
                v.tensor_copy(sh1[:, 1:32], S1[:, 0:31])
                nc.sync.dma_start(sh1[1:128, 0:1], S1[0:127, 31:32])
                for t in range(16):
                    v.tensor_tensor(S0[:, t::16], S0[:, t::16], sh1[:], ALU.add)
                sh0 = sp.tile([P, 512], F32, tag="sh0")
                gp.memset(sh0[0:1, 0:1], 0.0)
                v.tensor_copy(sh0[:, 1:512], S0[:, 0:511])
                nc.sync.dma_start(sh0[1:128, 0:1], S0[0:127, 511:512])
                for t in range(16):
                    v.tensor_tensor(X[:, t::16], X[:, t::16], sh0[:], ALU.add)

            # P4: wrap + sinc in ROW-MAJOR (elementwise), tooth written
            # in-place into X (f32) block by block
            JB = 2048
            with tc.tile_pool(name="p4", bufs=1) as p4p:
                for jb in range(4):
                    c0 = jb * JB
                    xsl = X[:, c0:c0 + JB]
                    fsl = f0up[:, c0:c0 + JB]
                    A = p4p.tile([P, JB], F32, tag="A")
                    Bt = p4p.tile([P, JB], F32, tag="B")
                    Ct = p4p.tile([P, JB], F32, tag="C")
                    Dt = p4p.tile([P, JB], F32, tag="D")
                    v.tensor_scalar(A[:], xsl, MAGIC, None, ALU.add)
                    v.tensor_scalar(A[:], A[:], MAGIC, None, ALU.subtract)
                    v.tensor_tensor(A[:], xsl, A[:], ALU.subtract)            # xw
                    v.tensor_scalar(Bt[:], fsl, 0.001, None, ALU.add)         # den
                    v.reciprocal_approx_accurate(Ct[:], Bt[:], scratch=Dt[:])
                    v.tensor_scalar(A[:], A[:], 44100.0, None, ALU.mult)      # num
                    v.tensor_tensor(A[:], A[:], Ct[:], ALU.mult)              # v
                    v.tensor_scalar(Bt[:], A[:], 0.5, None, ALU.mult)
                    v.tensor_scalar(Bt[:], Bt[:], MAGIC, None, ALU.add)
                    v.tensor_scalar(Bt[:], Bt[:], MAGIC, None, ALU.subtract)
                    v.scalar_tensor_tensor(Bt[:], Bt[:], -2.0, A[:],
                                           ALU.mult, ALU.add)                 # w
                    sc.activation(Ct[:], Bt[:], AF.Sin, scale=PI)             # sin(pi w)
                    v.tensor_scalar(Bt[:], A[:], PI, None, ALU.mult)          # pi v
                    v.tensor_scalar(Dt[:], Bt[:], 0.0, None, ALU.is_equal)    # m
                    v.tensor_tensor(Bt[:], Bt[:], Dt[:], ALU.add)             # piv + m
                    v.tensor_tensor(Ct[:], Ct[:], Dt[:], ALU.add)             # sin + m
                    v.reciprocal_approx_accurate(A[:], Bt[:], scratch=Dt[:])
                    v.tensor_tensor(xsl, Ct[:], A[:], ALU.mult)               # tooth

            # P3: transpose tooth to column-major f32r
            for m in range(64):
                pt = ps_tr.tile([P, P], F32, tag="tr")
                nc.tensor.transpose(pt[:], X[:, m * P:(m + 1) * P], eye[:])
                sc.copy(xt[:, 4 + m:4 + m + 64 * (P - 1) + 1:64], pt[:])
        # fx_pool closed: f0up/X freed

    # ---------------- F: forward DFT, filter, inverse DFT, OLA ----------
    with ExitStack() as fctx:
        fcons = fctx.enter_context(tc.tile_pool(name="fcons", bufs=1))
        fstr = fctx.enter_context(tc.tile_pool(name="fstream", bufs=1))
        olap = fctx.enter_context(tc.tile_pool(name="ola", bufs=2))
        wc = fcons.tile([P, 8 * NBP], F32R, tag="wc")
        nc.sync.dma_start(wc[:], wc_d[:].bitcast(F32R))
        wss = fcons.tile([P, 8 * NBP], F32R, tag="ws")
        nc.sync.dma_start(wss[:], ws_d[:].bitcast(F32R))
        vc = fcons.tile([P, 10 * BS], F32R, tag="vc")
        nc.sync.dma_start(vc[:], vc_d[:].bitcast(F32R))
        vs = fcons.tile([P, 10 * BS], F32R, tag="vs")
        nc.sync.dma_start(vs[:], vs_d[:].bitcast(F32R))

        out_v = out_d.rearrange("(t p r) -> t p r", t=16, p=P)
        prev_h1 = None
        prev_h2 = None
        NB = ((0, 258), (258, 256))
        for mt in range(MT):
            M = P if mt < 16 else 1
            r0, r1 = (mt * P, mt * P + M) if mt < 16 else (NF - 1, NF)
            hmt = fstr.tile([M, NBINS], F32, tag="hm")
            hpt = fstr.tile([M, NBINS], F32, tag="hp")
            nmt = fstr.tile([M, NBINS], F32, tag="nm")
            nc.sync.dma_start(hmt[:], hm_d[r0:r1, :])
            nc.sync.dma_start(hpt[:], hp_d[r0:r1, :])
            nc.sync.dma_start(nmt[:], nm_d[r0:r1, :])
            ee = fstr.tile([M, NBINS], F32, tag="ee")
            sc.activation(ee[:], hmt[:], AF.Exp)
            en = fstr.tile([M, NBINS], F32, tag="en")
            sc.activation(en[:], nmt[:], AF.Exp)
            zz = fstr.tile([M, NBINS], F32, tag="zz")
            v.tensor_scalar(zz[:], hpt[:], 0.5, None, ALU.mult)
            v.tensor_scalar(zz[:], zz[:], MAGIC, None, ALU.add)
            v.tensor_scalar(zz[:], zz[:], MAGIC, None, ALU.subtract)
            nc.vector.scalar_tensor_tensor(zz[:], zz[:], -2.0, hpt[:],
                                           ALU.mult, ALU.add)
            ssn = fstr.tile([M, NBINS], F32, tag="ssn")
            sc.activation(ssn[:], zz[:], AF.Sin, scale=PI)
            v.tensor_scalar(zz[:], zz[:], 0.5, None, ALU.add)
            mg = fstr.tile([M, NBINS], F32, tag="mg")
            v.tensor_scalar(mg[:], zz[:], 1.0, None, ALU.is_gt)
            nc.vector.scalar_tensor_tensor(zz[:], mg[:], -2.0, zz[:],
                                           ALU.mult, ALU.add)
            ccn = fstr.tile([M, NBINS], F32, tag="ccn")
            sc.activation(ccn[:], zz[:], AF.Sin, scale=PI)
            fR = fstr.tile([M, NBINS], F32, tag="fR")
            gp.tensor_tensor(fR[:], ee[:], ccn[:], ALU.mult)
            fI = fstr.tile([M, NBINS], F32, tag="fI")
            gp.tensor_tensor(fI[:], ee[:], ssn[:], ALU.mult)

            xr = fstr.tile([M, NBINS], F32, tag="xr")
            xi = fstr.tile([M, NBINS], F32, tag="xi")
            tmpc = fstr.tile([M, NBINS], F32, tag="tmpc")
            for sig_i, sig_t in ((0, xt), (1, noise_cm)):
                for b0, bw in NB:
                    cw = min(b0 + bw, NBINS) - b0   # clip to real bins
                    osl = slice(b0, b0 + cw)
                    pf = ps_fw.tile([M, 258], F32, tag="fw")
                    for ks in range(8):
                        lhs = sig_t[:, 512 * mt + ks:512 * mt + ks + 4 * (M - 1) + 1:4]
                        nc.tensor.matmul(
                            pf[:, 0:bw], lhs,
                            wc[:, ks * NBP + b0:ks * NBP + b0 + bw],
                            start=(ks == 0), stop=(ks == 7))
                    if sig_i == 0:
                        v.tensor_tensor(xr[:, osl], pf[:, 0:cw], fR[:, osl], ALU.mult)
                        v.tensor_tensor(xi[:, osl], pf[:, 0:cw], fI[:, osl], ALU.mult)
                    else:
                        v.tensor_tensor(tmpc[:, osl], pf[:, 0:cw], en[:, osl], ALU.mult)
                        v.tensor_tensor(xr[:, osl], xr[:, osl], tmpc[:, osl], ALU.add)
                    pfs = ps_fw.tile([M, 258], F32, tag="fw")
                    for ks in range(8):
                        lhs = sig_t[:, 512 * mt + ks:512 * mt + ks + 4 * (M - 1) + 1:4]
                        nc.tensor.matmul(
                            pfs[:, 0:bw], lhs,
                            wss[:, ks * NBP + b0:ks * NBP + b0 + bw],
                            start=(ks == 0), stop=(ks == 7))
                    if sig_i == 0:
                        v.tensor_tensor(tmpc[:, osl], pfs[:, 0:cw], fI[:, osl], ALU.mult)
                        v.tensor_tensor(xr[:, osl], xr[:, osl], tmpc[:, osl],
                                        ALU.subtract)
                        v.tensor_tensor(tmpc[:, osl], pfs[:, 0:cw], fR[:, osl], ALU.mult)
                        v.tensor_tensor(xi[:, osl], xi[:, osl], tmpc[:, osl], ALU.add)
                    else:
                        v.tensor_tensor(tmpc[:, osl], pfs[:, 0:cw], en[:, osl], ALU.mult)
                        v.tensor_tensor(xi[:, osl], xi[:, osl], tmpc[:, osl], ALU.add)

            # transpose XR, XI -> [bins(pad 640), frames]
            xrt = fstr.tile([P, 5 * P], F32R, tag="xrt")
            xit = fstr.tile([P, 5 * P], F32R, tag="xit")
            v.tensor_copy(xrt[:, 512:640], zeros128[:])
            v.tensor_copy(xit[:, 512:640], zeros128[:])
            for kb in range(4):
                pt = ps_tr.tile([P, P], F32, tag="tr")
                nc.tensor.transpose(pt[0:P, 0:M], xr[:, kb * P:(kb + 1) * P],
                                    eye[0:M, 0:M])
                sc.copy(xrt[:, kb * P:kb * P + M], pt[0:P, 0:M])
                pt2 = ps_tr.tile([P, P], F32, tag="tr")
                nc.tensor.transpose(pt2[0:P, 0:M], xi[:, kb * P:(kb + 1) * P],
                                    eye[0:M, 0:M])
                sc.copy(xit[:, kb * P:kb * P + M], pt2[0:P, 0:M])
            ptn = ps_tr.tile([P, P], F32, tag="tr")
            nc.tensor.transpose(ptn[0:1, 0:M], xr[:, 512:513], eye[0:M, 0:M])
            sc.copy(xrt[0:1, 512:512 + M], ptn[0:1, 0:M])
            ptn2 = ps_tr.tile([P, P], F32, tag="tr")
            nc.tensor.transpose(ptn2[0:1, 0:M], xi[:, 512:513], eye[0:M, 0:M])
            sc.copy(xit[0:1, 512:512 + M], ptn2[0:1, 0:M])

            # inverse matmuls: F[frames, 1024] in two PSUM halves
            Fh = []
            for h in range(2):
                pv = ps_iv.tile([M, BS], F32, tag="iv")
                for kb in range(5):
                    nc.tensor.matmul(
                        pv[:], xrt[:, kb * P:kb * P + M],
                        vc[:, kb * 2 * BS + h * BS:kb * 2 * BS + (h + 1) * BS],
                        start=(kb == 0), stop=False)
                    nc.tensor.matmul(
                        pv[:], xit[:, kb * P:kb * P + M],
                        vs[:, kb * 2 * BS + h * BS:kb * 2 * BS + (h + 1) * BS],
                        start=False, stop=(kb == 4))
                Fh.append(pv)

            # overlap-add
            h1f = olap.tile([M, BS], F32, tag="h1f")
            sc.copy(h1f[:], Fh[0][:])
            if mt > 0:
                nc.sync.dma_start(prev_h1[127:128, :], h1f[0:1, :])
                ob = olap.tile([P, BS], F32, tag="ob")
                v.tensor_tensor(ob[:], prev_h1[:], prev_h2[:], ALU.add)
                nc.sync.dma_start(out_v[mt - 1], ob[:])
            if mt < 16:
                h1s = olap.tile([P, BS], F32, tag="h1s")
                h2s = olap.tile([P, BS], F32, tag="h2s")
                nc.sync.dma_start(h1s[0:127, :], h1f[1:128, :])
                sc.copy(h2s[:], Fh[1][:])
                prev_h1, prev_h2 = h1s, h2s


def _build():
    if "nc" in _CACHE:
        return _CACHE["nc"]
    nc = bacc.Bacc("TRN2", target_bir_lowering=False, debug=False, num_devices=8)
    f0_d = nc.dram_tensor("f0", [NF], F32, kind="ExternalInput").ap()
    hm_d = nc.dram_tensor("hm", [NF, NBINS], F32, kind="ExternalInput").ap()
    hp_d = nc.dram_tensor("hp", [NF, NBINS], F32, kind="ExternalInput").ap()
    nm_d = nc.dram_tensor("nm", [NF, NBINS], F32, kind="ExternalInput").ap()
    nu_d = nc.dram_tensor("nu", [S], F32, kind="ExternalInput").ap()
    wc_d = nc.dram_tensor("wcc", [P, 8 * NBP], F32, kind="ExternalInput").ap()
    ws_d = nc.dram_tensor("wsc", [P, 8 * NBP], F32, kind="ExternalInput").ap()
    vc_d = nc.dram_tensor("vcc", [P, 10 * BS], F32, kind="ExternalInput").ap()
    vs_d = nc.dram_tensor("vsc", [P, 10 * BS], F32, kind="ExternalInput").ap()
    eye_d = nc.dram_tensor("eye", [P, P], F32, kind="ExternalInput").ap()
    out_d = nc.dram_tensor("out", [S], F32, kind="ExternalOutput").ap()
    with tile.TileContext(nc) as tc:
        with ExitStack() as ctx:
            _emit(nc, tc, ctx, (f0_d, hm_d, hp_d, nm_d, nu_d,
                                wc_d, ws_d, vc_d, vs_d, eye_d, out_d))
    nc.compile()
    _CACHE["nc"] = nc
    return nc


def kernel(**inputs):
    f0 = np.ascontiguousarray(np.asarray(inputs["f0_frames"], np.float32))
    hm = np.ascontiguousarray(np.asarray(inputs["harmonic_magnitude"], np.float32))
    hp = np.ascontiguousarray(np.asarray(inputs["harmonic_phase"], np.float32))
    nm = np.ascontiguousarray(np.asarray(inputs["noise_magnitude"], np.float32))
    nu = np.ascontiguousarray(np.asarray(inputs["noise_u"], np.float32))
    assert int(inputs["sampling_rate"]) == 44100 and int(inputs["block_size"]) == BS
    assert f0.shape == (8, NF) and nu.shape == (8, S)

    wc_dev, ws_dev, vc_dev, vs_dev, eye = _constants()
    nc = _build()
    in_maps = []
    for b in range(8):
        in_maps.append({
            "f0": f0[b], "hm": hm[b], "hp": hp[b], "nm": nm[b], "nu": nu[b],
            "wcc": wc_dev, "wsc": ws_dev, "vcc": vc_dev, "vsc": vs_dev, "eye": eye,
        })
    res = run_bass_kernel_spmd(nc, in_maps, list(range(8)), trace=TRACE)
    LAST_RES["res"] = res
    out = np.stack([res.results[b]["out"] for b in range(8)]).astype(np.float32)
    return out
